# revision 10
# baseline (speedup 1.0000x reference)
"""Chamfer distance L2 kernel for Trainium2 (8 NeuronCores).

Problem: B=32, N=M=4096, C=3 point clouds.
    D[b,n,m] = ||xyz1[b,n] - xyz2[b,m]||^2
    out[b]   = mean_n min_m D + mean_m min_n D

Strategy (per core, data-parallel over batch: 4 batches/core):
  - Augmented matmul trick: with xt = [x0,x1,x2, -0.5*||x||^2, 1] (K=5)
    and yt = [y0,y1,y2, 1, -0.5*||y||^2], the PE matmul computes
    S[n,m] = xt.T @ yt = x.y - 0.5||x||^2 - 0.5||y||^2 = -D[n,m]/2.
    So min_m D = -2 * max_m S  (all reductions become max over S).
  - fp16 hi/lo split-GEMM folded into the contraction dim (K=15,
    blocks X=[h,h,l] x Y=[h,l,h]) gives near-fp32 precision at fp16 PE
    speed; matmul cost is K-independent.  The lo*lo block is dropped.
  - The augmented matrices are assembled on the HOST (numpy): the device
    receives xt/yt [B_LOC, 15, 4096] fp16 directly.  This removes all
    on-device prep (wide loads, hi/lo splits, ~35 small gather DMAs per
    batch) and shrinks startup to two 120KB DMAs.
  - Post-matmul work per 128x4096 S-tile, balanced across 3 engines:
      * ACT drains columns [0, A_COLS) fp32->fp16 (1 elem/cycle@1.2GHz):
        pA1/pA2/pB1 fully + the first AB2 columns of pB2.
      * DVE drains the tail of pB2 via tensor_scalar(max) with fused
        accum_out row-max, then a 4x-mode fused row-max over the
        ACT-drained region and a 2x-mode tensor_tensor max accumulation
        (col direction) over [0, C1).
      * Pool (GpSimd) handles the col direction for [C1, 4096) via two
        per-tile partition_all_reduce(max) calls (scrA tail and scrD); a
        tiny DMA per EG tiles stashes the result rows on partitions of a
        [32, C2] collector, and one channels=32 partition_all_reduce at
        batch end finishes the col-max.
  - Batch finalize (deferred into the next batch's tile loop): row
    partials merged (TT-max) + reduced (sum); col accumulators
    partition-reduced (AR broadcasts the result to all partitions) then
    summed with WIDE fp16 4x-mode fused accumulate ops (a [128, C]
    tensor_scalar is ~4x cheaper than the equivalent [1, C] row op; the
    128x redundancy is divided out in the final scale).  Final means via
    ones-matmul partition contraction.

  Scheduling notes:
  - Each ENGINE writes its own scr tile (ACT: scrA, DVE: scrD) to avoid
    false cross-engine WAW serialization (dependency tracking is
    tile-granular).
  - PSUM is split into four single-buffered 1024-col tiles: the
    PE->drain->PE reuse ring per psum tile is the pacing cycle.
  - A few dummy warmup matmuls at t=0 bring the PE out of its throttled
    p-state before the first real matmul.
  - Each batch's xt/yt input DMAs are issued from inside the previous
    batch's tile loop; each batch's finalize is deferred into the next
    batch's loop, so the in-order per-engine queues never head-of-line
    block at batch boundaries.
"""

import numpy as np

B_FULL = 32
N_CORES = 8
B_LOC = B_FULL // N_CORES  # 4
N = 4096
M = 4096
C = 3

I_TILES = N // 128  # 32 row tiles
K_AUG = 5
K20 = 3 * K_AUG  # 15: hi/lo split blocks (hh, hl, lh); lo*lo dropped

# ---- engine-split knobs (see module docstring) ----
AB2 = 144             # ACT's share of pB2's 1024 columns
A_COLS = 3072 + AB2   # total ACT-drained columns
D_COLS = M - A_COLS   # DVE fused drain+rowmax width (tail of pB2)
C1 = 2112             # DVE col-accum region [0, C1)
C2A = A_COLS - C1     # Pool col region within scrA
C2B = D_COLS          # Pool col region = scrD
C2 = C2A + C2B        # total Pool col width per tile
EG = 4                # tiles per collector-extract DMA group
DVE_ORDER = 1         # 0: dd before ca; 1: ca before dd
FIN_AT = 24           # tile index where the previous batch's finalize runs
PREP_AT = 18          # tile index where the next batch's input DMAs go
WARMUP_MMS = 8        # dummy matmuls at t=0 to exit the PE low p-state

# Lower bound for max reductions; true S values are > -100, and this stays
# representable in fp16.
NEG_BIG = -60000.0

INSTR_LABELS = {}


def _lab(ins, label):
    try:
        INSTR_LABELS[ins.ins.name] = label
    except Exception:
        pass
    return ins


def _build_bass():
    import concourse.bacc as bacc
    import concourse.mybir as mybir
    import concourse.tile as tile
    from concourse import bass_isa

    f32 = mybir.dt.float32
    f16 = mybir.dt.float16
    AL = mybir.AluOpType
    RMAX = bass_isa.ReduceOp.max

    nc = bacc.Bacc("TRN2", target_bir_lowering=False, debug=False)

    xt_d = nc.dram_tensor("xt", [B_LOC, K20, N], f16, kind="ExternalInput")
    yt_d = nc.dram_tensor("yt", [B_LOC, K20, M], f16, kind="ExternalInput")
    out = nc.dram_tensor("out", [1, B_LOC], f32, kind="ExternalOutput")

    with tile.TileContext(nc) as tc:
        with (
            tc.tile_pool(name="consts", bufs=1) as consts,
            tc.tile_pool(name="coords", bufs=2) as coords_pool,
            tc.tile_pool(name="scr", bufs=3) as scr_pool,
            tc.tile_pool(name="cacc", bufs=2) as cacc_pool,
            tc.tile_pool(name="rmax", bufs=2) as rmax_pool,
            tc.tile_pool(name="fin", bufs=2) as fin_pool,
            tc.tile_pool(name="psum", bufs=1, space="PSUM") as psum_pool,
        ):
            ones128 = consts.tile([128, 1], f32)
            nc.vector.memset(ones128, 1.0)
            warm16 = consts.tile([128, 512], f16)
            nc.vector.memset(warm16, 0.0)
            dummy = consts.tile([128, A_COLS], f16)
            # sums[:, 3b+0] = per-partition row-max partial sums (batch b)
            # sums[:, 3b+1] = col-sum partial from cacc1 (broadcast over 128p)
            # sums[:, 3b+2] = col-sum partial from cacc2a+2b (broadcast)
            sums = consts.tile([128, 3 * B_LOC], f32)
            nc.vector.memset(sums, 0.0)

            # PE p-state warmup: a burst of matmuls on zeroed data so the
            # cost model's ramp window elapses before the first real MM.
            pwarm = psum_pool.tile([128, 512], f32, tag="pA1")
            for _ in range(WARMUP_MMS):
                nc.tensor.matmul(
                    pwarm, lhsT=warm16[:, 0:128], rhs=warm16, start=True, stop=True
                )

            xts, yts = [], []

            def emit_loads(b):
                xt = coords_pool.tile([K20, N], f16, tag="xt", name=f"xt{b}")
                yt = coords_pool.tile([K20, M], f16, tag="yt", name=f"yt{b}")
                xts.append(xt)
                yts.append(yt)
                nc.sync.dma_start(out=yt, in_=yt_d[b])
                nc.sync.dma_start(out=xt, in_=xt_d[b])

            pending_finalize = [None]

            def emit_compute(b):
                xt, yt = xts[b], yts[b]
                cacc1 = cacc_pool.tile([128, C1], f16, tag="cacc1")
                collector = cacc_pool.tile([32, C2], f16, tag="coll")
                rowmaxA = rmax_pool.tile([128, I_TILES], f32, tag="rmA")
                rowmaxB = rmax_pool.tile([128, I_TILES], f32, tag="rmB")
                colscr = None

                for i in range(I_TILES):
                    # Deferred work from neighbors rides inside this loop so
                    # per-engine queues never stall at batch boundaries.
                    if i == FIN_AT and pending_finalize[0] is not None:
                        pending_finalize[0]()
                        pending_finalize[0] = None
                    if i == PREP_AT and b + 1 < B_LOC:
                        emit_loads(b + 1)

                    scrA = scr_pool.tile([128, A_COLS], f16, tag="scrA")
                    scrD = scr_pool.tile([128, D_COLS], f16, tag="scrD")

                    def em_mm(tag, width, col0):
                        pt = psum_pool.tile([128, width], f32, tag=tag)
                        for j2 in range(0, width, 512):
                            w = min(512, width - j2)
                            j0 = col0 + j2
                            _lab(
                                nc.tensor.matmul(
                                    pt[:, j2 : j2 + w],
                                    lhsT=xt[:, i * 128 : (i + 1) * 128],
                                    rhs=yt[:, j0 : j0 + w],
                                    start=True,
                                    stop=True,
                                ),
                                f"mm.{tag}.t{i}",
                            )
                        return pt

                    def em_dd(ptB):
                        # DVE fused drain + row-max of pB2's tail
                        _lab(nc.vector.tensor_scalar(
                            scrD[:],
                            ptB[:, AB2:1024],
                            NEG_BIG,
                            None,
                            AL.max,
                            AL.max,
                            accum_out=rowmaxB[:, i : i + 1],
                        ), f"dd.t{i}")

                    def em_ca():
                        # DVE col-accum over [0, C1)
                        if i == 0:
                            _lab(nc.vector.tensor_copy(cacc1[:], scrA[:, 0:C1]),
                                 "ca.t0")
                        else:
                            _lab(nc.vector.tensor_tensor(
                                cacc1[:], cacc1[:], scrA[:, 0:C1], AL.max
                            ), f"ca.t{i}")

                    def em_rm1():
                        _lab(nc.vector.tensor_scalar(
                            dummy[:],
                            scrA[:],
                            NEG_BIG,
                            None,
                            AL.max,
                            AL.max,
                            accum_out=rowmaxA[:, i : i + 1],
                        ), f"rm1.t{i}")

                    ptA1 = em_mm("pA1", 2048, 0)
                    _lab(nc.scalar.copy(scrA[:, 0:2048], ptA1[:]), f"dA1.t{i}")
                    ptA2 = em_mm("pA2", 1024, 2048)
                    ptB2 = em_mm("pB2", 1024, 3072)
                    if DVE_ORDER == 0:
                        em_dd(ptB2)
                    _lab(nc.scalar.copy(scrA[:, 2048:3072], ptA2[:]), f"dA2.t{i}")
                    if AB2:
                        _lab(nc.scalar.copy(
                            scrA[:, 3072:A_COLS], ptB2[:, 0:AB2]
                        ), f"dB2a.t{i}")
                    if DVE_ORDER == 0:
                        em_ca()
                    else:
                        em_ca()
                        em_dd(ptB2)
                    em_rm1()
                    # Pool col direction for [C1, 4096): per-tile partition
                    # max into a slot of the grouped scratch; one DMA per EG
                    # tiles stashes the EG result rows on collector
                    # partitions [i-EG+1, i].
                    s = i % EG
                    if s == 0:
                        colscr = cacc_pool.tile([128, EG * C2], f16, tag="colscr")
                    _lab(nc.gpsimd.partition_all_reduce(
                        colscr[:, s * C2 : s * C2 + C2A],
                        scrA[:, C1:A_COLS],
                        128,
                        RMAX,
                    ), f"ar1.t{i}")
                    _lab(nc.gpsimd.partition_all_reduce(
                        colscr[:, s * C2 + C2A : (s + 1) * C2],
                        scrD[:],
                        128,
                        RMAX,
                    ), f"ar2.t{i}")
                    if i == I_TILES - 3 and s == 1:
                        # early half-extract so the final group's collector
                        # chain (extract -> colC reduce) is shorter
                        nc.sync.dma_start(
                            out=collector[i - 1 : i + 1, :],
                            in_=colscr[0:1, 0 : 2 * C2],
                        )
                    elif i == I_TILES - 1:
                        nc.sync.dma_start(
                            out=collector[i - 1 : i + 1, :],
                            in_=colscr[0:1, 2 * C2 : 4 * C2],
                        )
                    elif s == EG - 1:
                        g0 = i - (EG - 1)
                        nc.sync.dma_start(
                            out=collector[g0 : g0 + EG, :],
                            in_=colscr[0:1, 0 : EG * C2],
                        )

                # ---- per-batch reductions (deferred into the next batch) ----
                def finalize():
                    rm = rmax_pool.tile([128, I_TILES], f32, tag="rm")
                    nc.vector.tensor_tensor(rm, rowmaxA, rowmaxB, AL.max)
                    nc.vector.tensor_reduce(
                        out=sums[:, 3 * b : 3 * b + 1],
                        in_=rm,
                        axis=mybir.AxisListType.X,
                        op=AL.add,
                    )
                    # Cross-partition max of the col accumulators; AR output
                    # is broadcast to all partitions, so the per-column sum
                    # can run as a cheap wide 4x op (128x redundancy divided
                    # out in the final scale).
                    c1r = fin_pool.tile([128, C1], f16, tag="c1r")
                    nc.gpsimd.partition_all_reduce(c1r[:], cacc1[:], 128, RMAX)
                    nc.vector.tensor_scalar(
                        dummy[:, 0:C1],
                        c1r[:],
                        NEG_BIG,
                        None,
                        AL.max,
                        AL.add,
                        accum_out=sums[:, 3 * b + 1 : 3 * b + 2],
                    )
                    colC = fin_pool.tile([32, C2], f16, tag="colC")
                    nc.gpsimd.partition_all_reduce(
                        colC[0:32, :], collector[0:32, :], 32, RMAX
                    )
                    nc.vector.tensor_scalar(
                        dummy[0:32, 0:C2],
                        colC[0:32, :],
                        NEG_BIG,
                        None,
                        AL.max,
                        AL.add,
                        accum_out=sums[0:32, 3 * b + 2 : 3 * b + 3],
                    )

                pending_finalize[0] = finalize

            emit_loads(0)
            for b in range(B_LOC):
                emit_compute(b)
            pending_finalize[0]()

            # ---- final: contract partitions via ones-matmul ----
            # ps_fin[0, 3b+k]: k=0 row partial (x1), k=1 cacc1 col partial
            # (broadcast over 128 partitions -> x128), k=2 collector col
            # partial (on 32 partitions -> x32).
            ps_fin = psum_pool.tile([1, 3 * B_LOC], f32, tag="pA2")
            nc.tensor.matmul(ps_fin, lhsT=ones128, rhs=sums, start=True, stop=True)
            allp = fin_pool.tile([1, 3 * B_LOC], f32)
            nc.scalar.copy(allp, ps_fin)
            # tot[b] = -2/4096 * (v0 + v1/128 + v2/32)
            tot = fin_pool.tile([1, B_LOC], f32)
            t2 = fin_pool.tile([1, B_LOC], f32)
            v = allp.rearrange("o (b k) -> o b k", k=3)
            nc.vector.tensor_scalar_mul(tot, v[:, :, 1], 1.0 / 128.0)
            nc.vector.tensor_scalar_mul(t2, v[:, :, 2], 1.0 / 32.0)
            nc.vector.tensor_add(tot, tot, t2)
            nc.vector.tensor_add(tot, tot, v[:, :, 0])
            nc.vector.tensor_scalar_mul(tot, tot, -2.0 / 4096.0)
            nc.sync.dma_start(out=out[:, :], in_=tot)

    nc.compile()
    return nc


_NC_CACHE = {}


def _get_nc():
    if "nc" not in _NC_CACHE:
        _NC_CACHE["nc"] = _build_bass()
    return _NC_CACHE["nc"]


def _augment(pts: np.ndarray, is_x: bool) -> np.ndarray:
    """Build the hi/lo-split augmented matrix [K20, npts] fp16 from
    [npts, 3] fp32 points.  Blocks X=[h,h,l], Y=[h,l,h]; within a block:
    rows 0-2 coords, one row -0.5*||p||^2, one row the constant
    (1 for hi blocks, 0 for lo blocks).  Mirrors the numerics the
    baseline computed on-device (all IEEE f32/f16 round-to-nearest)."""
    npts = pts.shape[0]
    w = np.ascontiguousarray(pts.T.astype(np.float32))          # [3, npts]
    wh = w.astype(np.float16)
    wl = (w - wh.astype(np.float32)).astype(np.float16)
    sq = (-0.5 * (w * w).sum(axis=0, dtype=np.float32)).astype(np.float32)
    sqh = sq.astype(np.float16)
    sql = (sq - sqh.astype(np.float32)).astype(np.float16)

    t = np.zeros((K20, npts), dtype=np.float16)
    sq_row, const_row = (3, 4) if is_x else (4, 3)
    blocks = "hhl" if is_x else "hlh"
    for rep, blk in enumerate(blocks):
        base = rep * K_AUG
        hi = blk == "h"
        t[base : base + 3] = wh if hi else wl
        t[base + sq_row] = sqh if hi else sql
        t[base + const_row] = 1.0 if hi else 0.0
    return t


def kernel(xyz1: np.ndarray, xyz2: np.ndarray) -> np.ndarray:
    from concourse.bass_utils import run_bass_kernel_spmd

    nc = _get_nc()
    xyz1 = np.asarray(xyz1, dtype=np.float32)
    xyz2 = np.asarray(xyz2, dtype=np.float32)
    xt = np.stack([_augment(xyz1[b], True) for b in range(B_FULL)])
    yt = np.stack([_augment(xyz2[b], False) for b in range(B_FULL)])
    in_maps = [
        {
            "xt": xt[c * B_LOC : (c + 1) * B_LOC],
            "yt": yt[c * B_LOC : (c + 1) * B_LOC],
        }
        for c in range(N_CORES)
    ]
    res = run_bass_kernel_spmd(nc, in_maps, core_ids=list(range(N_CORES)))
    out = np.concatenate([r["out"].reshape(B_LOC) for r in res.results])
    return out.astype(np.float32)


if __name__ == "__main__":
    rng = np.random.default_rng(0)
    a = rng.standard_normal((B_FULL, N, C), dtype=np.float32)
    b = rng.standard_normal((B_FULL, M, C), dtype=np.float32)
    r = kernel(a, b)
    print(r)


# revision 18
# speedup vs baseline: 1.0684x; 1.0684x over previous
"""Chamfer distance L2 kernel for Trainium2 (8 NeuronCores).

Problem: B=32, N=M=4096, C=3 point clouds.
    D[b,n,m] = ||xyz1[b,n] - xyz2[b,m]||^2
    out[b]   = mean_n min_m D + mean_m min_n D

Strategy (per core, data-parallel over batch: 4 batches/core):
  - Augmented matmul trick: with xt = [x0,x1,x2, -0.5*||x||^2, 1] (K=5)
    and yt = [y0,y1,y2, 1, -0.5*||y||^2], the PE matmul computes
    S[n,m] = xt.T @ yt = x.y - 0.5||x||^2 - 0.5||y||^2 = -D[n,m]/2.
    So min_m D = -2 * max_m S  (all reductions become max over S).
  - fp16 hi/lo split-GEMM folded into the contraction dim (K=15,
    blocks X=[h,h,l] x Y=[h,l,h]) gives near-fp32 precision at fp16 PE
    speed; matmul cost is K-independent.  The lo*lo block is dropped.
  - The augmented matrices are assembled on the HOST (numpy): the device
    receives xt/yt [B_LOC, 15, 4096] fp16 directly.  This removes all
    on-device prep (wide loads, hi/lo splits, ~35 small gather DMAs per
    batch) and shrinks startup to two 120KB DMAs.
  - Post-matmul work per 128x4096 S-tile, balanced across 3 engines:
      * ACT drains columns [0, A_COLS) fp32->fp16 (1 elem/cycle@1.2GHz):
        pA1/pA2/pB1 fully + the first AB2 columns of pB2.
      * DVE drains the tail of pB2 via tensor_scalar(max) with fused
        accum_out row-max, then a 4x-mode fused row-max over the
        ACT-drained region and a 2x-mode tensor_tensor max accumulation
        (col direction) over [0, C1).
      * Pool (GpSimd) handles the col direction for [C1, 4096) via two
        per-tile partition_all_reduce(max) calls (scrA tail and scrD); a
        tiny DMA per EG tiles stashes the result rows on partitions of a
        [32, C2] collector, and one channels=32 partition_all_reduce at
        batch end finishes the col-max.
  - Batch finalize (deferred into the next batch's tile loop): row
    partials merged (TT-max) + reduced (sum); col accumulators
    partition-reduced (AR broadcasts the result to all partitions) then
    summed with WIDE fp16 4x-mode fused accumulate ops (a [128, C]
    tensor_scalar is ~4x cheaper than the equivalent [1, C] row op; the
    128x redundancy is divided out in the final scale).  Final means via
    ones-matmul partition contraction.

  Scheduling notes:
  - Each ENGINE writes its own scr tile (ACT: scrA, DVE: scrD) to avoid
    false cross-engine WAW serialization (dependency tracking is
    tile-granular).
  - PSUM is split into four single-buffered 1024-col tiles: the
    PE->drain->PE reuse ring per psum tile is the pacing cycle.
  - A few dummy warmup matmuls at t=0 bring the PE out of its throttled
    p-state before the first real matmul.
  - Each batch's xt/yt input DMAs are issued from inside the previous
    batch's tile loop; each batch's finalize is deferred into the next
    batch's loop, so the in-order per-engine queues never head-of-line
    block at batch boundaries.
"""

import numpy as np

B_FULL = 32
N_CORES = 8
B_LOC = B_FULL // N_CORES  # 4
N = 4096
M = 4096
C = 3

I_TILES = N // 128  # 32 row tiles
K_AUG = 5
K20 = 3 * K_AUG  # 15: hi/lo split blocks (hh, hl, lh); lo*lo dropped

# ---- engine-split knobs (see module docstring) ----
AB2 = 0               # ACT's share of pB2's 1024 columns
A_COLS = 3072 + AB2   # total ACT-drained columns
D_COLS = M - A_COLS   # DVE fused drain+rowmax width (tail of pB2)
C1 = 2048             # DVE col-accum region [0, C1)
C2A = A_COLS - C1     # Pool col region within scrA
C2B = D_COLS          # Pool col region = scrD
C2 = C2A + C2B        # total Pool col width per tile
EG = 4                # tiles per collector-extract DMA group
DVE_ORDER = 1         # 0: dd before ca; 1: ca before dd
FIN_AT = 24           # tile index where the previous batch's finalize runs
PREP_AT = 18          # tile index where the next batch's input DMAs go
WARMUP_MMS = 8        # dummy matmuls at t=0 to exit the PE low p-state

# Lower bound for max reductions; true S values are > -100, and this stays
# representable in fp16.
NEG_BIG = -60000.0

INSTR_LABELS = {}


def _lab(ins, label):
    try:
        INSTR_LABELS[ins.ins.name] = label
    except Exception:
        pass
    return ins


def _build_bass():
    import concourse.bacc as bacc
    import concourse.mybir as mybir
    import concourse.tile as tile
    from concourse import bass_isa

    f32 = mybir.dt.float32
    f16 = mybir.dt.float16
    AL = mybir.AluOpType
    RMAX = bass_isa.ReduceOp.max

    nc = bacc.Bacc("TRN2", target_bir_lowering=False, debug=False)

    xt_d = nc.dram_tensor("xt", [B_LOC, K20, N], f16, kind="ExternalInput")
    yt_d = nc.dram_tensor("yt", [B_LOC, K20, M], f16, kind="ExternalInput")
    out = nc.dram_tensor("out", [1, B_LOC], f32, kind="ExternalOutput")

    with tile.TileContext(nc) as tc:
        with (
            tc.tile_pool(name="consts", bufs=1) as consts,
            tc.tile_pool(name="coords", bufs=2) as coords_pool,
            tc.tile_pool(name="scr", bufs=3) as scr_pool,
            tc.tile_pool(name="cacc", bufs=2) as cacc_pool,
            tc.tile_pool(name="rmax", bufs=2) as rmax_pool,
            tc.tile_pool(name="fin", bufs=2) as fin_pool,
            tc.tile_pool(name="psum", bufs=1, space="PSUM") as psum_pool,
        ):
            ones128 = consts.tile([128, 1], f32)
            nc.vector.memset(ones128, 1.0)
            warm16 = consts.tile([128, 512], f16)
            nc.vector.memset(warm16, 0.0)
            dummy = consts.tile([128, A_COLS], f16)
            # sums[:, 3b+0] = per-partition row-max partial sums (batch b)
            # sums[:, 3b+1] = col-sum partial from cacc1 (broadcast over 128p)
            # sums[:, 3b+2] = col-sum partial from cacc2a+2b (broadcast)
            sums = consts.tile([128, 3 * B_LOC], f32)
            nc.vector.memset(sums, 0.0)

            # PE p-state warmup: a burst of matmuls on zeroed data so the
            # cost model's ramp window elapses before the first real MM.
            pwarm = psum_pool.tile([128, 512], f32, tag="pA1")
            for _ in range(WARMUP_MMS):
                nc.tensor.matmul(
                    pwarm, lhsT=warm16[:, 0:128], rhs=warm16, start=True, stop=True
                )

            xts, yts = [], []

            def emit_loads(b):
                xt = coords_pool.tile([K20, N], f16, tag="xt", name=f"xt{b}")
                yt = coords_pool.tile([K20, M], f16, tag="yt", name=f"yt{b}")
                xts.append(xt)
                yts.append(yt)
                if b == 0:
                    # first tile's pA1 matmuls only need xt + yt[:, 0:1024]
                    nc.sync.dma_start(out=xt, in_=xt_d[b])
                    nc.sync.dma_start(out=yt[:, 0:1024], in_=yt_d[b][:, 0:1024])
                    nc.sync.dma_start(out=yt[:, 1024:M], in_=yt_d[b][:, 1024:M])
                else:
                    nc.sync.dma_start(out=yt, in_=yt_d[b])
                    nc.sync.dma_start(out=xt, in_=xt_d[b])

            pending_finalize = [None]

            def emit_compute(b):
                xt, yt = xts[b], yts[b]
                cacc1 = cacc_pool.tile([128, C1], f16, tag="cacc1")
                collector = cacc_pool.tile([32, C2], f16, tag="coll")
                rowmaxA = rmax_pool.tile([128, I_TILES], f32, tag="rmA")
                rowmaxB = rmax_pool.tile([128, I_TILES], f32, tag="rmB")
                colscr = None
                deferred_rm1 = []

                for i in range(I_TILES):
                    # Deferred work from neighbors rides inside this loop so
                    # per-engine queues never stall at batch boundaries.  The
                    # finalize is staged over several tiles so its Pool/DVE
                    # bursts don't stall the steady-state drains.
                    if pending_finalize[0] is not None and i >= FIN_AT:
                        stage = i - FIN_AT
                        steps = pending_finalize[0]
                        if stage < len(steps):
                            steps[stage]()
                        if stage == len(steps) - 1:
                            pending_finalize[0] = None
                    if i == PREP_AT and b + 1 < B_LOC:
                        emit_loads(b + 1)

                    scrA = scr_pool.tile([128, A_COLS], f16, tag="scrA")
                    scrD = scr_pool.tile([128, D_COLS], f16, tag="scrD")

                    def em_mm(tag, width, col0):
                        pt = psum_pool.tile([128, width], f32, tag=tag)
                        for j2 in range(0, width, 512):
                            w = min(512, width - j2)
                            j0 = col0 + j2
                            _lab(
                                nc.tensor.matmul(
                                    pt[:, j2 : j2 + w],
                                    lhsT=xt[:, i * 128 : (i + 1) * 128],
                                    rhs=yt[:, j0 : j0 + w],
                                    start=True,
                                    stop=True,
                                ),
                                f"mm.{tag}.t{i}",
                            )
                        return pt

                    def em_dd(ptB):
                        # DVE fused drain + row-max of pB2's tail
                        _lab(nc.vector.tensor_scalar(
                            scrD[:],
                            ptB[:, AB2:1024],
                            NEG_BIG,
                            None,
                            AL.max,
                            AL.max,
                            accum_out=rowmaxB[:, i : i + 1],
                        ), f"dd.t{i}")

                    def em_ca():
                        # DVE col-accum over [0, C1)
                        if i == 0:
                            _lab(nc.vector.tensor_copy(cacc1[:], scrA[:, 0:C1]),
                                 "ca.t0")
                        else:
                            _lab(nc.vector.tensor_tensor(
                                cacc1[:], cacc1[:], scrA[:, 0:C1], AL.max
                            ), f"ca.t{i}")

                    def em_rm1():
                        _lab(nc.vector.tensor_scalar(
                            dummy[:],
                            scrA[:],
                            NEG_BIG,
                            None,
                            AL.max,
                            AL.max,
                            accum_out=rowmaxA[:, i : i + 1],
                        ), f"rm1.t{i}")

                    ptA1 = em_mm("pA1", 1024, 0)
                    _lab(nc.scalar.copy(scrA[:, 0:1024], ptA1[:]), f"dA1.t{i}")
                    ptA2 = em_mm("pA2", 1024, 1024)
                    _lab(nc.scalar.copy(scrA[:, 1024:2048], ptA2[:]), f"dA2.t{i}")
                    ptB1 = em_mm("pB1", 1024, 2048)
                    ptB2 = em_mm("pB2", 1024, 3072)
                    # Last two tiles of the last batch: dd/ca first (they
                    # feed the finalize chains) and rm1 deferred past tile
                    # 31 (it only feeds the cheap row merge; scr bufs=3
                    # keeps the deferred tiles alive).
                    tailish = b == B_LOC - 1 and i >= I_TILES - 2
                    if DVE_ORDER == 0 or tailish:
                        em_dd(ptB2)
                    _lab(nc.scalar.copy(scrA[:, 2048:3072], ptB1[:]), f"dB.t{i}")
                    if AB2:
                        _lab(nc.scalar.copy(
                            scrA[:, 3072:A_COLS], ptB2[:, 0:AB2]
                        ), f"dB2a.t{i}")
                    if DVE_ORDER == 0 or tailish:
                        em_ca()
                    else:
                        em_ca()
                        em_dd(ptB2)
                    if tailish:
                        deferred_rm1.append(em_rm1)
                        if i == I_TILES - 1:
                            for f in deferred_rm1:
                                f()
                    else:
                        em_rm1()
                    # Pool col direction for [C1, 4096): per-tile partition
                    # max into a slot of the grouped scratch; one DMA per EG
                    # tiles stashes the EG result rows on collector
                    # partitions [i-EG+1, i].
                    s = i % EG
                    if s == 0:
                        colscr = cacc_pool.tile([128, EG * C2], f16, tag="colscr")
                    _lab(nc.gpsimd.partition_all_reduce(
                        colscr[:, s * C2 : s * C2 + C2A],
                        scrA[:, C1:A_COLS],
                        128,
                        RMAX,
                    ), f"ar1.t{i}")
                    _lab(nc.gpsimd.partition_all_reduce(
                        colscr[:, s * C2 + C2A : (s + 1) * C2],
                        scrD[:],
                        128,
                        RMAX,
                    ), f"ar2.t{i}")
                    # The last group is extracted in two halves so the final
                    # chain (extract -> colC reduce) is shorter.
                    lg0 = I_TILES - EG
                    if i == I_TILES - 1:
                        nc.sync.dma_start(
                            out=collector[lg0 + EG // 2 : I_TILES, :],
                            in_=colscr[0:1, (EG // 2) * C2 : EG * C2],
                        )
                    elif i == lg0 + EG // 2 - 1:
                        nc.sync.dma_start(
                            out=collector[lg0 : lg0 + EG // 2, :],
                            in_=colscr[0:1, 0 : (EG // 2) * C2],
                        )
                    elif s == EG - 1:
                        g0 = i - (EG - 1)
                        nc.sync.dma_start(
                            out=collector[g0 : g0 + EG, :],
                            in_=colscr[0:1, 0 : EG * C2],
                        )

                # ---- per-batch reductions (staged into the next batch) ----
                c1r = fin_pool.tile([128, C1], f16, tag="c1r")
                colC = fin_pool.tile([32, C2], f16, tag="colC")

                def fin_rows():
                    rm = rmax_pool.tile([128, I_TILES], f32, tag="rm")
                    nc.vector.tensor_tensor(rm, rowmaxA, rowmaxB, AL.max)
                    nc.vector.tensor_reduce(
                        out=sums[:, 3 * b : 3 * b + 1],
                        in_=rm,
                        axis=mybir.AxisListType.X,
                        op=AL.add,
                    )

                def fin_c1_ar():
                    nc.gpsimd.partition_all_reduce(c1r[:], cacc1[:], 128, RMAX)

                def fin_c1_sum():
                    # AR output is broadcast to all partitions, so the
                    # per-column sum runs as a cheap wide 4x op (the 128x
                    # redundancy is divided out in the final scale).
                    nc.vector.tensor_scalar(
                        dummy[:, 0:C1],
                        c1r[:],
                        NEG_BIG,
                        None,
                        AL.max,
                        AL.add,
                        accum_out=sums[:, 3 * b + 1 : 3 * b + 2],
                    )

                def fin_colC_ar():
                    nc.gpsimd.partition_all_reduce(
                        colC[0:32, :], collector[0:32, :], 32, RMAX
                    )

                def fin_colC_sum():
                    nc.vector.tensor_scalar(
                        dummy[0:32, 0:C2],
                        colC[0:32, :],
                        NEG_BIG,
                        None,
                        AL.max,
                        AL.add,
                        accum_out=sums[0:32, 3 * b + 2 : 3 * b + 3],
                    )

                pending_finalize[0] = [
                    fin_c1_ar, fin_rows, fin_c1_sum, fin_colC_ar, fin_colC_sum,
                ]

            emit_loads(0)
            for b in range(B_LOC):
                emit_compute(b)
            for step in pending_finalize[0]:
                step()

            # ---- final: contract partitions via ones-matmul ----
            # ps_fin[0, 3b+k]: k=0 row partial (x1), k=1 cacc1 col partial
            # (broadcast over 128 partitions -> x128), k=2 collector col
            # partial (on 32 partitions -> x32).
            ps_fin = psum_pool.tile([1, 3 * B_LOC], f32, tag="pA2")
            nc.tensor.matmul(ps_fin, lhsT=ones128, rhs=sums, start=True, stop=True)
            allp = fin_pool.tile([1, 3 * B_LOC], f32)
            nc.scalar.copy(allp, ps_fin)
            # tot[b] = -2/4096 * (v0 + v1/128 + v2/32)
            tot = fin_pool.tile([1, B_LOC], f32)
            t2 = fin_pool.tile([1, B_LOC], f32)
            v = allp.rearrange("o (b k) -> o b k", k=3)
            nc.vector.tensor_scalar_mul(tot, v[:, :, 1], 1.0 / 128.0)
            nc.vector.tensor_scalar_mul(t2, v[:, :, 2], 1.0 / 32.0)
            nc.vector.tensor_add(tot, tot, t2)
            nc.vector.tensor_add(tot, tot, v[:, :, 0])
            nc.vector.tensor_scalar_mul(tot, tot, -2.0 / 4096.0)
            nc.sync.dma_start(out=out[:, :], in_=tot)

    nc.compile()
    return nc


_NC_CACHE = {}


def _get_nc():
    if "nc" not in _NC_CACHE:
        _NC_CACHE["nc"] = _build_bass()
    return _NC_CACHE["nc"]


def _augment(pts: np.ndarray, is_x: bool) -> np.ndarray:
    """Build the hi/lo-split augmented matrix [K20, npts] fp16 from
    [npts, 3] fp32 points.  Blocks X=[h,h,l], Y=[h,l,h]; within a block:
    rows 0-2 coords, one row -0.5*||p||^2, one row the constant
    (1 for hi blocks, 0 for lo blocks).  Mirrors the numerics the
    baseline computed on-device (all IEEE f32/f16 round-to-nearest)."""
    npts = pts.shape[0]
    w = np.ascontiguousarray(pts.T.astype(np.float32))          # [3, npts]
    wh = w.astype(np.float16)
    wl = (w - wh.astype(np.float32)).astype(np.float16)
    sq = (-0.5 * (w * w).sum(axis=0, dtype=np.float32)).astype(np.float32)
    sqh = sq.astype(np.float16)
    sql = (sq - sqh.astype(np.float32)).astype(np.float16)

    t = np.zeros((K20, npts), dtype=np.float16)
    sq_row, const_row = (3, 4) if is_x else (4, 3)
    blocks = "hhl" if is_x else "hlh"
    for rep, blk in enumerate(blocks):
        base = rep * K_AUG
        hi = blk == "h"
        t[base : base + 3] = wh if hi else wl
        t[base + sq_row] = sqh if hi else sql
        t[base + const_row] = 1.0 if hi else 0.0
    return t


def kernel(xyz1: np.ndarray, xyz2: np.ndarray) -> np.ndarray:
    from concourse.bass_utils import run_bass_kernel_spmd

    nc = _get_nc()
    xyz1 = np.asarray(xyz1, dtype=np.float32)
    xyz2 = np.asarray(xyz2, dtype=np.float32)
    xt = np.stack([_augment(xyz1[b], True) for b in range(B_FULL)])
    yt = np.stack([_augment(xyz2[b], False) for b in range(B_FULL)])
    in_maps = [
        {
            "xt": xt[c * B_LOC : (c + 1) * B_LOC],
            "yt": yt[c * B_LOC : (c + 1) * B_LOC],
        }
        for c in range(N_CORES)
    ]
    res = run_bass_kernel_spmd(nc, in_maps, core_ids=list(range(N_CORES)))
    out = np.concatenate([r["out"].reshape(B_LOC) for r in res.results])
    return out.astype(np.float32)


if __name__ == "__main__":
    rng = np.random.default_rng(0)
    a = rng.standard_normal((B_FULL, N, C), dtype=np.float32)
    b = rng.standard_normal((B_FULL, M, C), dtype=np.float32)
    r = kernel(a, b)
    print(r)
